# revision 29
# baseline (speedup 1.0000x reference)
"""GNN message-passing kernel for trn2 (8 NeuronCores, SPMD, 4 launches).

Algorithm restructuring vs the reference:
  - logmap0 + W_up + leaky_relu per node (sharded by node across cores).
  - round A: sum_z[d] = sum_e (u @ W_pl)[src_e], sum_w[d] = sum_e (u @ W_lw[64:])[src_e]
    (linearity: the tiny matmuls commute with segment_sum), via per-edge 12B
    gathers + per-partition prefix scans + boundary differences.
    sel = (relu(z1) - relu(z0) > logit(T)).
  - round B: s2[d] = sum_e (sel*u@W_lw[:64])[src_e] likewise; wsel = sigmoid(s2+sum_w);
    g = wsel*sel per node; u3 = g*u (bf16 table).
  - round C: a_x[d] = relu(sum_e u3[src_e]) via 128B bf16 row gathers + one-hot
    matmul segment reduction per 128-dst block; out = proj(expmap0(u + a_x)).
Host does index preprocessing only (sharding, sorting, padding).
"""
import os
import sys

sys.path.insert(0, "/opt/trn_rl_repo")

import numpy as np
import ml_dtypes

import concourse.bacc as bacc
import concourse.bass as bass
import concourse.tile as tile
import concourse.mybir as mybir
from concourse import bass_utils
from concourse.masks import make_identity

F32 = mybir.dt.float32
BF16 = mybir.dt.bfloat16
I32 = mybir.dt.int32
ALU = mybir.AluOpType
ACT = mybir.ActivationFunctionType

NC_N = 8
NSH = 12500
PPD = 98                  # dsts per partition
NPAD = 128 * PPD          # 12544 padded nodes per core
N_ALL = NC_N * NPAD       # 100352
ZROW = N_ALL              # zero row index in pack tables
NBLK = NPAD // 128        # 98 dst blocks per core
MIN_NORM = 1e-15
ATANH_CLIP = 1.0 - 1e-7
PROJ_MAXN = 1.0 - 4e-3
SEL_THR = float(np.log(np.float64(0.48) / np.float64(0.52)))  # logit threshold


# ---------------------------------------------------------------- host prep
def _binpack(counts_d, n_bins, cap):
    """Assign the NPAD dst ids to n_bins bins (exactly cap ids each),
    balancing total edge count per bin. Returns assign[n_bins, cap]."""
    import heapq
    order = np.argsort(-counts_d, kind="stable")
    heap = [(0, b) for b in range(n_bins)]
    heapq.heapify(heap)
    assign = np.empty((n_bins, cap), np.int64)
    fill = np.zeros(n_bins, np.int64)
    for dst in order:
        load, b = heapq.heappop(heap)
        assign[b, fill[b]] = dst
        fill[b] += 1
        if fill[b] < cap:
            heapq.heappush(heap, (load + int(counts_d[dst]), b))
    return assign


def host_prep(edge_index):
    """Index prep with load-balanced assignments at every level (bin-packing)
    to minimize gather-instruction counts: nodes->cores (balances per-core
    edge totals), then dsts->partitions (KA) and dsts->blocks (KC). Device
    outputs come back in permuted layouts; kernel() un/re-permutes on host."""
    import heapq
    src = np.asarray(edge_index[0], dtype=np.int64)
    dst = np.asarray(edge_index[1], dtype=np.int64)
    n_nodes = NC_N * NSH
    indeg = np.bincount(dst, minlength=n_nodes)
    norder = np.argsort(-indeg, kind="stable")
    heap = [(0, c) for c in range(NC_N)]
    heapq.heapify(heap)
    fill = np.zeros(NC_N, np.int64)
    node_core = np.empty(n_nodes, np.int64)
    node_pos = np.empty(n_nodes, np.int64)
    for n in norder:
        load, c = heapq.heappop(heap)
        node_core[n] = c
        node_pos[n] = fill[c]
        fill[c] += 1
        if fill[c] < NSH:
            heapq.heappush(heap, (load + int(indeg[n]), c))
    perm = node_core * NPAD + node_pos          # node id -> padded table row
    nodes_by_core_pos = np.empty((NC_N, NSH), np.int64)
    nodes_by_core_pos[node_core, node_pos] = np.arange(n_nodes)
    srcp = perm[src]
    dstp = perm[dst]
    cores = []
    KA = 0
    KC = 0
    for c in range(NC_N):
        m = (dstp // NPAD) == c
        s = srcp[m]
        d = dstp[m] - c * NPAD
        order = np.argsort(d, kind="stable")
        s_o = s[order]
        counts_d = np.bincount(d, minlength=NPAD)
        dstarts = np.zeros(NPAD + 1, np.int64)
        dstarts[1:] = np.cumsum(counts_d)
        sigmaA = _binpack(counts_d, 128, PPD)       # [128, PPD] dst ids
        sigmaC = _binpack(counts_d, NBLK, 128)      # [98, 128] dst ids
        counts_pj = counts_d[sigmaA]                # [128, PPD]
        counts_bq = counts_d[sigmaC]                # [98, 128]
        KA = max(KA, int(counts_pj.sum(1).max()) + 1)
        KC = max(KC, int(np.ceil(counts_bq.sum(1).max() / 128)))
        cores.append(dict(s_o=s_o, dstarts=dstarts, sigmaA=sigmaA,
                          sigmaC=sigmaC, counts_pj=counts_pj,
                          counts_bq=counts_bq))
    KA = int(np.ceil(KA / 4) * 4)
    KC = int(KC)
    for pc in cores:
        s_o, dstarts = pc["s_o"], pc["dstarts"]
        sigmaA, sigmaC = pc["sigmaA"], pc["sigmaC"]
        counts_pj, counts_bq = pc["counts_pj"], pc["counts_bq"]
        idxA = np.full((128, KA), ZROW, np.int32)
        for p in range(128):
            segs = [s_o[dstarts[t]:dstarts[t + 1]] for t in sigmaA[p]]
            row = np.concatenate(segs) if segs else np.empty(0, np.int64)
            idxA[p, 1:1 + len(row)] = row
        ends = np.zeros((128, PPD + 1), np.int64)
        ends[:, 1:] = np.cumsum(counts_pj, axis=1)
        bidx = (ends + np.arange(128)[:, None] * KA).astype(np.int32)
        # round C
        idxC = np.zeros((NBLK, 128, KC), np.int32)
        dstC = np.full((NBLK, 128, KC), 999.0, np.float32)
        for b in range(NBLK):
            segs = [s_o[dstarts[t]:dstarts[t + 1]] for t in sigmaC[b]]
            eb_s = (np.concatenate(segs) if segs else np.empty(0, np.int64))
            eb_d = np.repeat(np.arange(128), counts_bq[b])
            n = len(eb_s)
            idxC[b].T.flat[:n] = eb_s
            dstC[b].T.flat[:n] = eb_d
        pc["idxA"], pc["bidx"] = idxA, bidx
        pc["idxC"], pc["dstC"] = idxC, dstC
        pc["sigA_flat"] = sigmaA.reshape(-1)
        pc["sigC_flat"] = sigmaC.reshape(-1)
    return cores, KA, KC, nodes_by_core_pos


# ---------------------------------------------------------------- L1: stage 1
def build_L1():
    """Stage 1, restructured for short dependency chains:
      pass 1 (per block): load xT (host-pretransposed), Square -> sq,
        n2 col = sq^T @ ones (PE), psU = x @ Wup (PE, lhsT=xT), stash to ubuf.
      wide: s2 = artanh(clip(|x|))/|x| for all 98 blocks in one op chain.
      pass 2 (per block): u = lrelu(psU * s2) via one ACT op (scale=s2 col,
        alpha=0.01), pack = u @ Wcat via PE (lhsT=u^T), both DMA'd out.
    """
    nc = bacc.Bacc("TRN2", target_bir_lowering=False, debug=False, num_devices=NC_N)
    xT_in = nc.dram_tensor("xT", [128, NPAD], F32, kind="ExternalInput").ap()
    Wup = nc.dram_tensor("Wup", [128, 64], F32, kind="ExternalInput").ap()
    Wcat = nc.dram_tensor("Wcat", [64, 4], F32, kind="ExternalInput").ap()
    u_sh = nc.dram_tensor("u_sh", [NPAD, 64], F32, kind="ExternalOutput").ap()
    pack_sh = nc.dram_tensor("pack_sh", [NPAD, 4], F32, kind="ExternalOutput").ap()

    with tile.TileContext(nc) as tc:
        with tc.tile_pool(name="const", bufs=1) as cp, \
             tc.tile_pool(name="big", bufs=1) as bigp, \
             tc.tile_pool(name="sb", bufs=4) as sp, \
             tc.tile_pool(name="sc", bufs=2) as scp, \
             tc.tile_pool(name="ps", bufs=2, space="PSUM") as pp, \
             tc.tile_pool(name="psn", bufs=1, space="PSUM") as ppn:
            ident = cp.tile([128, 128], F32)
            make_identity(nc, ident[:])
            wu = cp.tile([128, 64], F32)
            nc.sync.dma_start(out=wu[:], in_=Wup[:])
            wc = cp.tile([64, 4], F32)
            nc.sync.dma_start(out=wc[:], in_=Wcat[:])
            ones = cp.tile([128, 1], F32)
            nc.vector.memset(ones[:], 1.0)

            ubuf = bigp.tile([128, NBLK * 64], F32)      # x @ Wup, pre-act
            psN2 = ppn.tile([128, NBLK], F32, space="PSUM")
            # one big xT load (per-block loads cost ~700ns each on Sync)
            xbig = bigp.tile([128, NPAD], F32)
            XG = NBLK // 7
            for g in range(7):
                nc.sync.dma_start(
                    out=xbig[:, g * XG * 128:(g + 1) * XG * 128],
                    in_=xT_in[:, g * XG * 128:(g + 1) * XG * 128])

            for b in range(NBLK):
                xt = xbig[:, b * 128:(b + 1) * 128]
                sq = sp.tile([128, 128], F32, tag="sq")
                nc.scalar.activation(out=sq[:], in_=xt, func=ACT.Square)
                nc.tensor.matmul(psN2[:, b:b + 1], lhsT=sq[:], rhs=ones[:],
                                 start=True, stop=True)
                psU = pp.tile([128, 64], F32, tag="psU", space="PSUM")
                nc.tensor.matmul(psU[:], lhsT=xt, rhs=wu[:], start=True,
                                 stop=True)
                nc.scalar.copy(out=ubuf[:, b * 64:(b + 1) * 64], in_=psU[:])

            # wide scalar chain: s2 = artanh(min(max(sqrt(n2),MIN),CLIP)) / nm
            n2 = scp.tile([128, NBLK], F32, tag="n2")
            nc.vector.tensor_copy(out=n2[:], in_=psN2[:])
            nv = scp.tile([128, NBLK], F32, tag="nv")
            nc.scalar.activation(out=nv[:], in_=n2[:], func=ACT.Sqrt)
            nm = scp.tile([128, NBLK], F32, tag="nm")
            nc.vector.tensor_scalar_max(nm[:], nv[:], MIN_NORM)
            cl = scp.tile([128, NBLK], F32, tag="cl")
            nc.vector.tensor_scalar_min(cl[:], nm[:], ATANH_CLIP)
            num = scp.tile([128, NBLK], F32, tag="num")
            nc.vector.tensor_scalar_add(num[:], cl[:], 1.0)
            den = scp.tile([128, NBLK], F32, tag="den")
            nc.vector.tensor_scalar(out=den[:], in0=cl[:], scalar1=-1.0,
                                    scalar2=1.0, op0=ALU.mult, op1=ALU.add)
            rden = scp.tile([128, NBLK], F32, tag="rden")
            nc.vector.reciprocal(rden[:], den[:])
            q = scp.tile([128, NBLK], F32, tag="q")
            nc.vector.tensor_tensor(out=q[:], in0=num[:], in1=rden[:], op=ALU.mult)
            lq = scp.tile([128, NBLK], F32, tag="lq")
            nc.scalar.activation(out=lq[:], in_=q[:], func=ACT.Ln)
            rnm = scp.tile([128, NBLK], F32, tag="rnm")
            nc.vector.reciprocal(rnm[:], nm[:])
            s1 = scp.tile([128, NBLK], F32, tag="s1")
            nc.vector.tensor_tensor(out=s1[:], in0=lq[:], in1=rnm[:], op=ALU.mult)
            s2 = scp.tile([128, NBLK], F32, tag="s2")
            nc.vector.tensor_scalar_mul(s2[:], s1[:], 0.5)

            ubig = bigp.tile([128, NBLK * 64], F32)   # scaled+activated u
            pkbig = bigp.tile([128, NBLK * 4], F32)
            u_view = u_sh.rearrange("(b p) f -> p b f", p=128)
            pk_view = pack_sh.rearrange("(b p) f -> p b f", p=128)
            UG = NBLK // 7
            for b in range(NBLK):
                u_b = ubig[:, b * 64:(b + 1) * 64]
                nc.scalar.activation(out=u_b, in_=ubuf[:, b * 64:(b + 1) * 64],
                                     func=ACT.Lrelu, scale=s2[:, b:b + 1],
                                     alpha=0.01)
                psUT = pp.tile([64, 128], F32, tag="psUT", space="PSUM")
                nc.tensor.transpose(psUT[:], u_b, ident[:])
                uT = sp.tile([64, 128], F32, tag="uT")
                nc.scalar.copy(out=uT[:], in_=psUT[:])
                psPn = pp.tile([128, 4], F32, tag="psPn", space="PSUM")
                nc.tensor.matmul(psPn[:], lhsT=uT[:], rhs=wc[:], start=True,
                                 stop=True)
                nc.vector.tensor_copy(out=pkbig[:, b * 4:(b + 1) * 4], in_=psPn[:])
                if (b + 1) % UG == 0:
                    g0 = b + 1 - UG
                    nc.sync.dma_start(
                        out=u_view[:, g0:b + 1, :],
                        in_=ubig[:, g0 * 64:(b + 1) * 64].rearrange(
                            "p (b f) -> p b f", f=64))
                    nc.sync.dma_start(
                        out=pk_view[:, g0:b + 1, :],
                        in_=pkbig[:, g0 * 4:(b + 1) * 4].rearrange(
                            "p (b f) -> p b f", f=4))
    nc.compile()
    return nc


# ---------------------------------------------------------------- L2: round A
def build_L2(KA, n_gather=4):
    nc = bacc.Bacc("TRN2", target_bir_lowering=False, debug=False, num_devices=NC_N)
    tab = nc.dram_tensor("pack1_tab", [N_ALL + 1, 3], F32, kind="ExternalInput").ap()
    idxA = nc.dram_tensor("idxA", [128, KA], I32, kind="ExternalInput").ap()
    bidx = nc.dram_tensor("bidx", [128, PPD + 1], I32, kind="ExternalInput").ap()
    a_in = nc.dram_tensor("a_in", [128, PPD], F32, kind="ExternalInput").ap()
    sel_o = nc.dram_tensor("sel_o", [128, PPD], F32, kind="ExternalOutput").ap()
    sumw_o = nc.dram_tensor("sumw_o", [128, PPD], F32, kind="ExternalOutput").ap()
    pack2_o = nc.dram_tensor("pack2_o", [128, PPD], F32, kind="ExternalOutput").ap()

    KAc = KA // n_gather
    with tile.TileContext(nc) as tc:
        with tc.tile_pool(name="sb", bufs=1) as sp, \
             tc.tile_pool(name="dram", bufs=1, space="DRAM") as dp:
            idx_t = sp.tile([128, KA], I32)
            nc.sync.dma_start(out=idx_t[:], in_=idxA[:])
            gp = sp.tile([128, KA * 3], F32)
            gp3 = gp[:].rearrange("p (k c) -> p k c", c=3)
            # HW vector-indirect DMA only honors [128,1] offsets (one
            # descriptor per partition); wider offset APs silently read
            # contiguous rows. One instruction per column it is.
            for k in range(KA):
                nc.gpsimd.indirect_dma_start(
                    out=gp[:, k * 3:(k + 1) * 3],
                    out_offset=None,
                    in_=tab[:],
                    in_offset=bass.IndirectOffsetOnAxis(
                        ap=idx_t[:, k:k + 1], axis=0),
                )
            cum = sp.tile([128, KA * 3], F32)
            cum3 = cum[:].rearrange("p (k c) -> p k c", c=3)
            for j in range(3):
                nc.vector.tensor_tensor_scan(
                    out=cum3[:, :, j], data0=gp3[:, :, j], data1=gp3[:, :, j],
                    initial=0.0, op0=ALU.add, op1=ALU.bypass)
            spill = dp.tile([128 * KA, 3], F32)
            nc.sync.dma_start(
                out=spill[:].rearrange("(p k) c -> p (k c)", p=128), in_=cum[:])
            bidx_t = sp.tile([128, PPD + 1], I32)
            nc.sync.dma_start(out=bidx_t[:], in_=bidx[:])
            bv = sp.tile([128, (PPD + 1) * 3], F32)
            for k in range(PPD + 1):
                nc.gpsimd.indirect_dma_start(
                    out=bv[:, k * 3:(k + 1) * 3], out_offset=None, in_=spill[:],
                    in_offset=bass.IndirectOffsetOnAxis(
                        ap=bidx_t[:, k:k + 1], axis=0),
                )
            sums = sp.tile([128, PPD * 3], F32)
            nc.vector.tensor_tensor(out=sums[:], in0=bv[:, 3:],
                                    in1=bv[:, :PPD * 3], op=ALU.subtract)
            s3 = sums[:].rearrange("p (k c) -> p k c", c=3)
            r0 = sp.tile([128, PPD], F32)
            nc.vector.tensor_scalar_max(r0[:], s3[:, :, 0], 0.0)
            r1 = sp.tile([128, PPD], F32)
            nc.vector.tensor_scalar_max(r1[:], s3[:, :, 1], 0.0)
            dd = sp.tile([128, PPD], F32)
            nc.vector.tensor_sub(dd[:], r1[:], r0[:])
            sel = sp.tile([128, PPD], F32)
            nc.vector.tensor_scalar(out=sel[:], in0=dd[:], scalar1=SEL_THR,
                                    scalar2=0.0, op0=ALU.is_gt)
            nc.sync.dma_start(out=sel_o[:], in_=sel[:])
            sumw = sp.tile([128, PPD], F32)
            nc.vector.tensor_copy(out=sumw[:], in_=s3[:, :, 2])
            nc.sync.dma_start(out=sumw_o[:], in_=sumw[:])
            a_t = sp.tile([128, PPD], F32)
            nc.sync.dma_start(out=a_t[:], in_=a_in[:])
            p2 = sp.tile([128, PPD], F32)
            nc.vector.tensor_tensor(out=p2[:], in0=sel[:], in1=a_t[:], op=ALU.mult)
            nc.sync.dma_start(out=pack2_o[:], in_=p2[:])
    nc.compile()
    return nc


# ---------------------------------------------------------------- L3: round B
def build_L3(KC):
    """Round B, block-structured: reuses L4's idxC/dstC tables; one-hot
    matmul segment sums replace scan+spill+boundary (saves 99 gathers)."""
    nc = bacc.Bacc("TRN2", target_bir_lowering=False, debug=False, num_devices=NC_N)
    tab = nc.dram_tensor("pack2_tab", [N_ALL + 1, 1], F32, kind="ExternalInput").ap()
    idxC = nc.dram_tensor("idxC", [NBLK, 128, KC], I32, kind="ExternalInput").ap()
    dstC = nc.dram_tensor("dstC", [NBLK, 128, KC], F32, kind="ExternalInput").ap()
    iota = nc.dram_tensor("iota", [128, 128], F32, kind="ExternalInput").ap()
    sumw_i = nc.dram_tensor("sumw_i", [128, NBLK], F32, kind="ExternalInput").ap()
    sel_i = nc.dram_tensor("sel_i", [128, NBLK], F32, kind="ExternalInput").ap()
    u_in = nc.dram_tensor("u_in", [NPAD, 64], F32, kind="ExternalInput").ap()
    u3_o = nc.dram_tensor("u3_o", [NPAD, 64], F32, kind="ExternalOutput").ap()

    OB = 8
    with tile.TileContext(nc) as tc:
        with tc.tile_pool(name="const", bufs=1) as cp, \
             tc.tile_pool(name="sb", bufs=3) as sp, \
             tc.tile_pool(name="sc", bufs=2) as scp, \
             tc.tile_pool(name="u", bufs=2) as up, \
             tc.tile_pool(name="ps", bufs=4, space="PSUM") as pp:
            iota_t = cp.tile([128, 128], F32)
            nc.sync.dma_start(out=iota_t[:], in_=iota[:])
            s2w = cp.tile([128, NBLK], F32)
            for b in range(NBLK):
                idx_t = sp.tile([128, KC], I32, tag="idx")
                nc.sync.dma_start(out=idx_t[:], in_=idxC[b])
                dst_t = sp.tile([128, KC], F32, tag="dst")
                nc.sync.dma_start(out=dst_t[:], in_=dstC[b])
                g = sp.tile([128, KC], F32, tag="g")
                for k in range(KC):
                    nc.gpsimd.indirect_dma_start(
                        out=g[:, k:k + 1], out_offset=None, in_=tab[:],
                        in_offset=bass.IndirectOffsetOnAxis(
                            ap=idx_t[:, k:k + 1], axis=0),
                    )
                S = sp.tile([128, KC * 128], F32, tag="S")
                Sv = S[:].rearrange("p (k d) -> p k d", d=128)
                for k0 in range(0, KC, OB):
                    kk = min(OB, KC - k0)
                    nc.vector.tensor_tensor(
                        out=Sv[:, k0:k0 + kk, :],
                        in0=dst_t[:, k0:k0 + kk].to_broadcast([128, kk, 128]),
                        in1=iota_t[:].unsqueeze(1).broadcast_to([128, kk, 128]),
                        op=ALU.is_equal)
                ps = pp.tile([128, 1], F32, tag="acc", space="PSUM")
                for k in range(KC):
                    nc.tensor.matmul(ps[:], lhsT=S[:, k * 128:(k + 1) * 128],
                                     rhs=g[:, k:k + 1],
                                     start=(k == 0), stop=(k == KC - 1))
                nc.vector.tensor_copy(out=s2w[:, b:b + 1], in_=ps[:])
            sumw_t = scp.tile([128, NBLK], F32, tag="sumw")
            nc.sync.dma_start(out=sumw_t[:], in_=sumw_i[:])
            zs = scp.tile([128, NBLK], F32, tag="zs")
            nc.vector.tensor_add(zs[:], s2w[:], sumw_t[:])
            wsel = scp.tile([128, NBLK], F32, tag="wsel")
            nc.scalar.activation(out=wsel[:], in_=zs[:], func=ACT.Sigmoid)
            sel_t = scp.tile([128, NBLK], F32, tag="sel")
            nc.sync.dma_start(out=sel_t[:], in_=sel_i[:])
            g2 = scp.tile([128, NBLK], F32, tag="g2")
            nc.vector.tensor_tensor(out=g2[:], in0=wsel[:], in1=sel_t[:],
                                    op=ALU.mult)
            STR = 14
            u_v = u_in.rearrange("(p j) f -> p j f", p=128)
            u3_v = u3_o.rearrange("(p j) f -> p j f", p=128)
            for s0 in range(0, NBLK, STR):
                ut = up.tile([128, STR * 64], F32, tag="ut")
                nc.sync.dma_start(out=ut[:], in_=u_v[:, s0:s0 + STR, :])
                u3t = up.tile([128, STR * 64], F32, tag="u3t")
                gb = g2[:, s0:s0 + STR].to_broadcast([128, STR, 64])
                nc.vector.tensor_tensor(
                    out=u3t[:].rearrange("p (j f) -> p j f", f=64),
                    in0=ut[:].rearrange("p (j f) -> p j f", f=64),
                    in1=gb, op=ALU.mult)
                nc.sync.dma_start(out=u3_v[:, s0:s0 + STR, :], in_=u3t[:])
    nc.compile()
    return nc


# ---------------------------------------------------------------- L4: round C
def build_L4(KC):
    nc = bacc.Bacc("TRN2", target_bir_lowering=False, debug=False, num_devices=NC_N)
    tab = nc.dram_tensor("u3_tab", [N_ALL, 64], F32, kind="ExternalInput").ap()
    u_in = nc.dram_tensor("u_in", [NPAD, 64], F32, kind="ExternalInput").ap()
    idxC = nc.dram_tensor("idxC", [NBLK, 128, KC], I32, kind="ExternalInput").ap()
    dstC = nc.dram_tensor("dstC", [NBLK, 128, KC], F32, kind="ExternalInput").ap()
    iota = nc.dram_tensor("iota", [128, 128], F32, kind="ExternalInput").ap()
    out_o = nc.dram_tensor("out_o", [NPAD, 64], F32, kind="ExternalOutput").ap()

    OB = 8  # one-hot batch (chunks per DVE op)
    with tile.TileContext(nc) as tc:
        with tc.tile_pool(name="const", bufs=1) as cp, \
             tc.tile_pool(name="sb", bufs=3) as sp, \
             tc.tile_pool(name="sc", bufs=3) as scp, \
             tc.tile_pool(name="ps", bufs=4, space="PSUM") as pp:
            iota_t = cp.tile([128, 128], F32)
            nc.sync.dma_start(out=iota_t[:], in_=iota[:])
            for b in range(NBLK):
                idx_t = sp.tile([128, KC], I32, tag="idx")
                nc.sync.dma_start(out=idx_t[:], in_=idxC[b])
                dst_t = sp.tile([128, KC], F32, tag="dst")
                nc.sync.dma_start(out=dst_t[:], in_=dstC[b])
                g = sp.tile([128, KC * 64], F32, tag="g")
                g3 = g[:].rearrange("p (k f) -> p k f", f=64)
                for k in range(KC):
                    nc.gpsimd.indirect_dma_start(
                        out=g3[:, k, :], out_offset=None, in_=tab[:],
                        in_offset=bass.IndirectOffsetOnAxis(ap=idx_t[:, k:k + 1], axis=0),
                    )
                S = sp.tile([128, KC * 128], F32, tag="S")
                Sv = S[:].rearrange("p (k d) -> p k d", d=128)
                for k0 in range(0, KC, OB):
                    kk = min(OB, KC - k0)
                    nc.vector.tensor_tensor(
                        out=Sv[:, k0:k0 + kk, :],
                        in0=dst_t[:, k0:k0 + kk].to_broadcast([128, kk, 128]),
                        in1=iota_t[:].unsqueeze(1).broadcast_to([128, kk, 128]),
                        op=ALU.is_equal)
                ps = pp.tile([128, 64], F32, tag="acc", space="PSUM")
                for k in range(KC):
                    nc.tensor.matmul(ps[:], lhsT=S[:, k * 128:(k + 1) * 128],
                                     rhs=g[:, k * 64:(k + 1) * 64],
                                     start=(k == 0), stop=(k == KC - 1))
                ut = sp.tile([128, 64], F32, tag="ut")
                nc.sync.dma_start(out=ut[:], in_=u_in[b * 128:(b + 1) * 128, :])
                ax = sp.tile([128, 64], F32, tag="ax")
                nc.vector.tensor_scalar_max(ax[:], ps[:], 0.0)
                o = sp.tile([128, 64], F32, tag="o")
                nc.vector.tensor_add(o[:], ut[:], ax[:])
                # expmap0 + proj
                sq = sp.tile([128, 64], F32, tag="sq")
                n2 = scp.tile([128, 1], F32, tag="n2")
                nc.scalar.activation(out=sq[:], in_=o[:], func=ACT.Square,
                                     accum_out=n2[:])
                nv = scp.tile([128, 1], F32, tag="nv")
                nc.scalar.activation(out=nv[:], in_=n2[:], func=ACT.Sqrt)
                nm = scp.tile([128, 1], F32, tag="nm")
                nc.vector.tensor_scalar_max(nm[:], nv[:], MIN_NORM)
                th = scp.tile([128, 1], F32, tag="th")
                nc.scalar.activation(out=th[:], in_=nm[:], func=ACT.Tanh)
                rn4 = scp.tile([128, 1], F32, tag="rn4")
                nc.vector.reciprocal(rn4[:], nm[:])
                f1 = scp.tile([128, 1], F32, tag="f1")
                nc.vector.tensor_tensor(out=f1[:], in0=th[:], in1=rn4[:],
                                        op=ALU.mult)
                # proj factor: min(maxn / tanh, 1)
                rt = scp.tile([128, 1], F32, tag="rt")
                nc.vector.reciprocal(rt[:], th[:])
                cap = scp.tile([128, 1], F32, tag="cap")
                nc.vector.tensor_scalar(out=cap[:], in0=rt[:], scalar1=PROJ_MAXN,
                                        scalar2=1.0, op0=ALU.mult, op1=ALU.min)
                f2 = scp.tile([128, 1], F32, tag="f2")
                nc.vector.tensor_tensor(out=f2[:], in0=f1[:], in1=cap[:],
                                        op=ALU.mult)
                oo = sp.tile([128, 64], F32, tag="oo")
                nc.vector.tensor_tensor(out=oo[:], in0=o[:],
                                        in1=f2[:].to_broadcast([128, 64]),
                                        op=ALU.mult)
                nc.sync.dma_start(out=out_o[b * 128:(b + 1) * 128, :], in_=oo[:])
    nc.compile()
    return nc


# ---------------------------------------------------------------- runner
def _run(nc, in_maps, trace):
    return bass_utils.run_bass_kernel_spmd(
        nc, in_maps, core_ids=list(range(NC_N)), trace=trace)


def kernel(x, edge_index, W_up, W_pl, W_lw, trace=None):
    if trace is None:
        trace = bool(int(os.environ.get("GNN_TRACE", "0")))
    if trace:
        bass_utils.upload_artifacts = lambda tmpdir: "/dev/null"

    x = np.asarray(x, np.float32)
    W_up = np.asarray(W_up, np.float32)
    W_pl = np.asarray(W_pl, np.float32)
    W_lw = np.asarray(W_lw, np.float32)
    cores, KA, KC, nodes_cp = host_prep(edge_index)
    exec_times = []

    # ---- L1
    Wcat = np.concatenate([W_pl, W_lw[64:128], W_lw[0:64]], axis=1)  # [64,4]
    xT_pad = np.zeros((NC_N, 128, NPAD), np.float32)
    for c in range(NC_N):
        xT_pad[c, :, :NSH] = x[nodes_cp[c]].T
    nc1 = build_L1()
    r1 = _run(nc1, [{"xT": xT_pad[c], "Wup": W_up, "Wcat": Wcat}
                    for c in range(NC_N)], trace)
    exec_times.append(r1.exec_time_ns)
    u_sh = [r1.results[c]["u_sh"] for c in range(NC_N)]
    pack_sh = [r1.results[c]["pack_sh"] for c in range(NC_N)]

    # ---- L2
    pack1_tab = np.concatenate(
        [np.concatenate([p[:, :3] for p in pack_sh], 0),
         np.zeros((1, 3), np.float32)], 0)
    nc2 = build_L2(KA)
    r2 = _run(nc2, [{"pack1_tab": pack1_tab,
                     "idxA": cores[c]["idxA"],
                     "bidx": cores[c]["bidx"],
                     "a_in": pack_sh[c][:, 3][cores[c]["sigmaA"]]}
                    for c in range(NC_N)], trace)
    exec_times.append(r2.exec_time_ns)
    sel = [r2.results[c]["sel_o"] for c in range(NC_N)]
    sumw = [r2.results[c]["sumw_o"] for c in range(NC_N)]
    pack2 = [r2.results[c]["pack2_o"] for c in range(NC_N)]

    # ---- L3  (block-structured; inputs/outputs in transposed-sigmaC layout)
    iota = np.tile(np.arange(128, dtype=np.float32)[None, :], (128, 1))
    p2_parts = []
    sigCT = []
    for c in range(NC_N):
        p2_full = np.zeros(NPAD, np.float32)
        p2_full[cores[c]["sigA_flat"]] = pack2[c].reshape(-1)
        p2_parts.append(p2_full)
        sigCT.append(cores[c]["sigC_flat"].reshape(NBLK, 128).T)  # [128, NBLK]
    pack2_tab = np.concatenate(
        [np.concatenate(p2_parts, 0), np.zeros(1, np.float32)], 0).reshape(-1, 1)

    def _toA(c, arr):  # sigmaA-layout [128, PPD] -> node vector
        full = np.zeros(NPAD, np.float32)
        full[cores[c]["sigA_flat"]] = arr.reshape(-1)
        return full

    nc3 = build_L3(KC)
    r3 = _run(nc3, [{"pack2_tab": pack2_tab,
                     "idxC": cores[c]["idxC"],
                     "dstC": cores[c]["dstC"],
                     "iota": iota,
                     "sumw_i": _toA(c, sumw[c])[sigCT[c]],
                     "sel_i": _toA(c, sel[c])[sigCT[c]],
                     "u_in": u_sh[c][sigCT[c].reshape(-1)]}
                    for c in range(NC_N)], trace)
    exec_times.append(r3.exec_time_ns)

    # ---- L4  (u3 rows come back in transposed-sigmaC order)
    u3_parts = []
    for c in range(NC_N):
        u3_full = np.zeros((NPAD, 64), np.float32)
        u3_full[sigCT[c].reshape(-1)] = r3.results[c]["u3_o"]
        u3_parts.append(u3_full)
    u3_tab = np.concatenate(u3_parts, 0)
    nc4 = build_L4(KC)
    r4 = _run(nc4, [{"u3_tab": u3_tab,
                     "u_in": u_sh[c][cores[c]["sigC_flat"]],
                     "idxC": cores[c]["idxC"],
                     "dstC": cores[c]["dstC"],
                     "iota": iota}
                    for c in range(NC_N)], trace)
    exec_times.append(r4.exec_time_ns)
    out = np.empty((NC_N * NSH, 64), np.float32)
    for c in range(NC_N):
        o_full = np.zeros((NPAD, 64), np.float32)
        o_full[cores[c]["sigC_flat"]] = r4.results[c]["out_o"]
        out[nodes_cp[c]] = o_full[:NSH]

    kernel.last_exec_times = exec_times
    return out



# revision 31
# speedup vs baseline: 1.1777x; 1.1777x over previous
"""GNN message-passing kernel for trn2 (8 NeuronCores, SPMD, 4 launches).

Algorithm restructuring vs the reference:
  - logmap0 + W_up + leaky_relu per node (sharded by node across cores).
  - round A: sum_z[d] = sum_e (u @ W_pl)[src_e], sum_w[d] = sum_e (u @ W_lw[64:])[src_e]
    (linearity: the tiny matmuls commute with segment_sum), via per-edge 12B
    gathers + per-partition prefix scans + boundary differences.
    sel = (relu(z1) - relu(z0) > logit(T)).
  - round B: s2[d] = sum_e (sel*u@W_lw[:64])[src_e] likewise; wsel = sigmoid(s2+sum_w);
    g = wsel*sel per node; u3 = g*u (bf16 table).
  - round C: a_x[d] = relu(sum_e u3[src_e]) via 128B bf16 row gathers + one-hot
    matmul segment reduction per 128-dst block; out = proj(expmap0(u + a_x)).
Host does index preprocessing only (sharding, sorting, padding).
"""
import os
import sys

sys.path.insert(0, "/opt/trn_rl_repo")

import numpy as np
import ml_dtypes

import concourse.bacc as bacc
import concourse.bass as bass
import concourse.tile as tile
import concourse.mybir as mybir
from concourse import bass_utils
from concourse.masks import make_identity

F32 = mybir.dt.float32
BF16 = mybir.dt.bfloat16
I32 = mybir.dt.int32
ALU = mybir.AluOpType
ACT = mybir.ActivationFunctionType

NC_N = 8
NSH = 12500
PPD = 98                  # dsts per partition
NPAD = 128 * PPD          # 12544 padded nodes per core
N_ALL = NC_N * NPAD       # 100352
ZROW = N_ALL              # zero row index in pack tables
NBLK = NPAD // 128        # 98 dst blocks per core
MIN_NORM = 1e-15
ATANH_CLIP = 1.0 - 1e-7
PROJ_MAXN = 1.0 - 4e-3
SEL_THR = float(np.log(np.float64(0.48) / np.float64(0.52)))  # logit threshold


# ---------------------------------------------------------------- host prep
def _binpack(counts_d, n_bins, cap):
    """Assign the NPAD dst ids to n_bins bins (exactly cap ids each),
    balancing total edge count per bin. Returns assign[n_bins, cap]."""
    import heapq
    order = np.argsort(-counts_d, kind="stable")
    heap = [(0, b) for b in range(n_bins)]
    heapq.heapify(heap)
    assign = np.empty((n_bins, cap), np.int64)
    fill = np.zeros(n_bins, np.int64)
    for dst in order:
        load, b = heapq.heappop(heap)
        assign[b, fill[b]] = dst
        fill[b] += 1
        if fill[b] < cap:
            heapq.heappush(heap, (load + int(counts_d[dst]), b))
    return assign


def host_prep(edge_index):
    """Index prep with load-balanced assignments at every level (bin-packing)
    to minimize gather-instruction counts: nodes->cores (balances per-core
    edge totals), then dsts->partitions (KA) and dsts->blocks (KC). Device
    outputs come back in permuted layouts; kernel() un/re-permutes on host."""
    import heapq
    src = np.asarray(edge_index[0], dtype=np.int64)
    dst = np.asarray(edge_index[1], dtype=np.int64)
    n_nodes = NC_N * NSH
    indeg = np.bincount(dst, minlength=n_nodes)
    norder = np.argsort(-indeg, kind="stable")
    heap = [(0, c) for c in range(NC_N)]
    heapq.heapify(heap)
    fill = np.zeros(NC_N, np.int64)
    node_core = np.empty(n_nodes, np.int64)
    node_pos = np.empty(n_nodes, np.int64)
    for n in norder:
        load, c = heapq.heappop(heap)
        node_core[n] = c
        node_pos[n] = fill[c]
        fill[c] += 1
        if fill[c] < NSH:
            heapq.heappush(heap, (load + int(indeg[n]), c))
    perm = node_core * NPAD + node_pos          # node id -> padded table row
    nodes_by_core_pos = np.empty((NC_N, NSH), np.int64)
    nodes_by_core_pos[node_core, node_pos] = np.arange(n_nodes)
    srcp = perm[src]
    dstp = perm[dst]
    cores = []
    KA = 0
    KC = 0
    for c in range(NC_N):
        m = (dstp // NPAD) == c
        s = srcp[m]
        d = dstp[m] - c * NPAD
        order = np.argsort(d, kind="stable")
        s_o = s[order]
        counts_d = np.bincount(d, minlength=NPAD)
        dstarts = np.zeros(NPAD + 1, np.int64)
        dstarts[1:] = np.cumsum(counts_d)
        sigmaA = _binpack(counts_d, 128, PPD)       # [128, PPD] dst ids
        sigmaC = _binpack(counts_d, NBLK, 128)      # [98, 128] dst ids
        counts_pj = counts_d[sigmaA]                # [128, PPD]
        counts_bq = counts_d[sigmaC]                # [98, 128]
        KA = max(KA, int(counts_pj.sum(1).max()) + 1)
        KC = max(KC, int(np.ceil(counts_bq.sum(1).max() / 128)))
        cores.append(dict(s_o=s_o, dstarts=dstarts, sigmaA=sigmaA,
                          sigmaC=sigmaC, counts_pj=counts_pj,
                          counts_bq=counts_bq))
    KA = int(np.ceil(KA / 4) * 4)
    KC = int(KC)
    for pc in cores:
        s_o, dstarts = pc["s_o"], pc["dstarts"]
        sigmaA, sigmaC = pc["sigmaA"], pc["sigmaC"]
        counts_pj, counts_bq = pc["counts_pj"], pc["counts_bq"]
        idxA = np.full((128, KA), ZROW, np.int32)
        for p in range(128):
            segs = [s_o[dstarts[t]:dstarts[t + 1]] for t in sigmaA[p]]
            row = np.concatenate(segs) if segs else np.empty(0, np.int64)
            idxA[p, 1:1 + len(row)] = row
        ends = np.zeros((128, PPD + 1), np.int64)
        ends[:, 1:] = np.cumsum(counts_pj, axis=1)
        bidx = (ends + np.arange(128)[:, None] * KA).astype(np.int32)
        # round C
        idxC = np.zeros((NBLK, 128, KC), np.int32)
        dstC = np.full((NBLK, 128, KC), 999.0, np.float32)
        for b in range(NBLK):
            segs = [s_o[dstarts[t]:dstarts[t + 1]] for t in sigmaC[b]]
            eb_s = (np.concatenate(segs) if segs else np.empty(0, np.int64))
            eb_d = np.repeat(np.arange(128), counts_bq[b])
            n = len(eb_s)
            idxC[b].T.flat[:n] = eb_s
            dstC[b].T.flat[:n] = eb_d
        pc["idxA"], pc["bidx"] = idxA, bidx
        pc["idxC"], pc["dstC"] = idxC, dstC
        pc["sigA_flat"] = sigmaA.reshape(-1)
        pc["sigC_flat"] = sigmaC.reshape(-1)
    return cores, KA, KC, nodes_by_core_pos


# ---------------------------------------------------------------- L1: stage 1
def build_L1():
    """Stage 1, restructured for short dependency chains:
      pass 1 (per block): load xT (host-pretransposed), Square -> sq,
        n2 col = sq^T @ ones (PE), psU = x @ Wup (PE, lhsT=xT), stash to ubuf.
      wide: s2 = artanh(clip(|x|))/|x| for all 98 blocks in one op chain.
      pass 2 (per block): u = lrelu(psU * s2) via one ACT op (scale=s2 col,
        alpha=0.01), pack = u @ Wcat via PE (lhsT=u^T), both DMA'd out.
    """
    nc = bacc.Bacc("TRN2", target_bir_lowering=False, debug=False, num_devices=NC_N)
    xT_in = nc.dram_tensor("xT", [128, NPAD], F32, kind="ExternalInput").ap()
    Wup = nc.dram_tensor("Wup", [128, 64], F32, kind="ExternalInput").ap()
    Wcat = nc.dram_tensor("Wcat", [64, 4], F32, kind="ExternalInput").ap()
    u_sh = nc.dram_tensor("u_sh", [NPAD, 64], F32, kind="ExternalOutput").ap()
    pack_sh = nc.dram_tensor("pack_sh", [NPAD, 4], F32, kind="ExternalOutput").ap()

    with tile.TileContext(nc) as tc:
        with tc.tile_pool(name="const", bufs=1) as cp, \
             tc.tile_pool(name="big", bufs=1) as bigp, \
             tc.tile_pool(name="sb", bufs=4) as sp, \
             tc.tile_pool(name="sc", bufs=2) as scp, \
             tc.tile_pool(name="ps", bufs=2, space="PSUM") as pp, \
             tc.tile_pool(name="psn", bufs=1, space="PSUM") as ppn:
            ident = cp.tile([128, 128], F32)
            make_identity(nc, ident[:])
            wu = cp.tile([128, 64], F32)
            nc.sync.dma_start(out=wu[:], in_=Wup[:])
            wc = cp.tile([64, 4], F32)
            nc.sync.dma_start(out=wc[:], in_=Wcat[:])
            ones = cp.tile([128, 1], F32)
            nc.vector.memset(ones[:], 1.0)

            ubuf = bigp.tile([128, NBLK * 64], F32)      # x @ Wup, pre-act
            psN2 = ppn.tile([128, NBLK], F32, space="PSUM")
            # one big xT load (per-block loads cost ~700ns each on Sync)
            xbig = bigp.tile([128, NPAD], F32)
            XG = NBLK // 7
            for g in range(7):
                nc.sync.dma_start(
                    out=xbig[:, g * XG * 128:(g + 1) * XG * 128],
                    in_=xT_in[:, g * XG * 128:(g + 1) * XG * 128])

            for b in range(NBLK):
                xt = xbig[:, b * 128:(b + 1) * 128]
                sq = sp.tile([128, 128], F32, tag="sq")
                nc.scalar.activation(out=sq[:], in_=xt, func=ACT.Square)
                nc.tensor.matmul(psN2[:, b:b + 1], lhsT=sq[:], rhs=ones[:],
                                 start=True, stop=True)
                psU = pp.tile([128, 64], F32, tag="psU", space="PSUM")
                nc.tensor.matmul(psU[:], lhsT=xt, rhs=wu[:], start=True,
                                 stop=True)
                nc.scalar.copy(out=ubuf[:, b * 64:(b + 1) * 64], in_=psU[:])

            # wide scalar chain: s2 = artanh(min(max(sqrt(n2),MIN),CLIP)) / nm
            n2 = scp.tile([128, NBLK], F32, tag="n2")
            nc.vector.tensor_copy(out=n2[:], in_=psN2[:])
            nv = scp.tile([128, NBLK], F32, tag="nv")
            nc.scalar.activation(out=nv[:], in_=n2[:], func=ACT.Sqrt)
            nm = scp.tile([128, NBLK], F32, tag="nm")
            nc.vector.tensor_scalar_max(nm[:], nv[:], MIN_NORM)
            cl = scp.tile([128, NBLK], F32, tag="cl")
            nc.vector.tensor_scalar_min(cl[:], nm[:], ATANH_CLIP)
            num = scp.tile([128, NBLK], F32, tag="num")
            nc.vector.tensor_scalar_add(num[:], cl[:], 1.0)
            den = scp.tile([128, NBLK], F32, tag="den")
            nc.vector.tensor_scalar(out=den[:], in0=cl[:], scalar1=-1.0,
                                    scalar2=1.0, op0=ALU.mult, op1=ALU.add)
            rden = scp.tile([128, NBLK], F32, tag="rden")
            nc.vector.reciprocal(rden[:], den[:])
            q = scp.tile([128, NBLK], F32, tag="q")
            nc.vector.tensor_tensor(out=q[:], in0=num[:], in1=rden[:], op=ALU.mult)
            lq = scp.tile([128, NBLK], F32, tag="lq")
            nc.scalar.activation(out=lq[:], in_=q[:], func=ACT.Ln)
            rnm = scp.tile([128, NBLK], F32, tag="rnm")
            nc.vector.reciprocal(rnm[:], nm[:])
            s1 = scp.tile([128, NBLK], F32, tag="s1")
            nc.vector.tensor_tensor(out=s1[:], in0=lq[:], in1=rnm[:], op=ALU.mult)
            s2 = scp.tile([128, NBLK], F32, tag="s2")
            nc.vector.tensor_scalar_mul(s2[:], s1[:], 0.5)

            ubig = bigp.tile([128, NBLK * 64], F32)   # scaled+activated u
            pkbig = bigp.tile([128, NBLK * 4], F32)
            u_view = u_sh.rearrange("(b p) f -> p b f", p=128)
            pk_view = pack_sh.rearrange("(b p) f -> p b f", p=128)
            UG = NBLK // 7
            for b in range(NBLK):
                u_b = ubig[:, b * 64:(b + 1) * 64]
                nc.scalar.activation(out=u_b, in_=ubuf[:, b * 64:(b + 1) * 64],
                                     func=ACT.Lrelu, scale=s2[:, b:b + 1],
                                     alpha=0.01)
                psUT = pp.tile([64, 128], F32, tag="psUT", space="PSUM")
                nc.tensor.transpose(psUT[:], u_b, ident[:])
                uT = sp.tile([64, 128], F32, tag="uT")
                nc.scalar.copy(out=uT[:], in_=psUT[:])
                psPn = pp.tile([128, 4], F32, tag="psPn", space="PSUM")
                nc.tensor.matmul(psPn[:], lhsT=uT[:], rhs=wc[:], start=True,
                                 stop=True)
                nc.vector.tensor_copy(out=pkbig[:, b * 4:(b + 1) * 4], in_=psPn[:])
                if (b + 1) % UG == 0:
                    g0 = b + 1 - UG
                    nc.sync.dma_start(
                        out=u_view[:, g0:b + 1, :],
                        in_=ubig[:, g0 * 64:(b + 1) * 64].rearrange(
                            "p (b f) -> p b f", f=64))
                    nc.sync.dma_start(
                        out=pk_view[:, g0:b + 1, :],
                        in_=pkbig[:, g0 * 4:(b + 1) * 4].rearrange(
                            "p (b f) -> p b f", f=4))
    nc.compile()
    return nc


# ---------------------------------------------------------------- L2: round A
def build_L2(KA, n_gather=4):
    nc = bacc.Bacc("TRN2", target_bir_lowering=False, debug=False, num_devices=NC_N)
    tab = nc.dram_tensor("pack1_tab", [N_ALL + 1, 3], F32, kind="ExternalInput").ap()
    idxA = nc.dram_tensor("idxA", [128, KA], I32, kind="ExternalInput").ap()
    bidx = nc.dram_tensor("bidx", [128, PPD + 1], I32, kind="ExternalInput").ap()
    a_in = nc.dram_tensor("a_in", [128, PPD], F32, kind="ExternalInput").ap()
    sel_o = nc.dram_tensor("sel_o", [128, PPD], F32, kind="ExternalOutput").ap()
    sumw_o = nc.dram_tensor("sumw_o", [128, PPD], F32, kind="ExternalOutput").ap()
    pack2_o = nc.dram_tensor("pack2_o", [128, PPD], F32, kind="ExternalOutput").ap()

    KAc = KA // n_gather
    with tile.TileContext(nc) as tc:
        with tc.tile_pool(name="sb", bufs=1) as sp, \
             tc.tile_pool(name="dram", bufs=1, space="DRAM") as dp:
            idx_t = sp.tile([128, KA], I32)
            nc.sync.dma_start(out=idx_t[:], in_=idxA[:])
            gp = sp.tile([128, KA * 3], F32)
            gp3 = gp[:].rearrange("p (k c) -> p k c", c=3)
            # HW vector-indirect DMA only honors [128,1] offsets (one
            # descriptor per partition); wider offset APs silently read
            # contiguous rows. One instruction per column it is.
            for k in range(KA):
                nc.gpsimd.indirect_dma_start(
                    out=gp[:, k * 3:(k + 1) * 3],
                    out_offset=None,
                    in_=tab[:],
                    in_offset=bass.IndirectOffsetOnAxis(
                        ap=idx_t[:, k:k + 1], axis=0),
                )
            cum = sp.tile([128, KA * 3], F32)
            cum3 = cum[:].rearrange("p (k c) -> p k c", c=3)
            for j in range(3):
                nc.vector.tensor_tensor_scan(
                    out=cum3[:, :, j], data0=gp3[:, :, j], data1=gp3[:, :, j],
                    initial=0.0, op0=ALU.add, op1=ALU.bypass)
            spill = dp.tile([128 * KA, 3], F32)
            nc.sync.dma_start(
                out=spill[:].rearrange("(p k) c -> p (k c)", p=128), in_=cum[:])
            bidx_t = sp.tile([128, PPD + 1], I32)
            nc.sync.dma_start(out=bidx_t[:], in_=bidx[:])
            bv = sp.tile([128, (PPD + 1) * 3], F32)
            for k in range(PPD + 1):
                nc.gpsimd.indirect_dma_start(
                    out=bv[:, k * 3:(k + 1) * 3], out_offset=None, in_=spill[:],
                    in_offset=bass.IndirectOffsetOnAxis(
                        ap=bidx_t[:, k:k + 1], axis=0),
                )
            sums = sp.tile([128, PPD * 3], F32)
            nc.vector.tensor_tensor(out=sums[:], in0=bv[:, 3:],
                                    in1=bv[:, :PPD * 3], op=ALU.subtract)
            s3 = sums[:].rearrange("p (k c) -> p k c", c=3)
            r0 = sp.tile([128, PPD], F32)
            nc.vector.tensor_scalar_max(r0[:], s3[:, :, 0], 0.0)
            r1 = sp.tile([128, PPD], F32)
            nc.vector.tensor_scalar_max(r1[:], s3[:, :, 1], 0.0)
            dd = sp.tile([128, PPD], F32)
            nc.vector.tensor_sub(dd[:], r1[:], r0[:])
            sel = sp.tile([128, PPD], F32)
            nc.vector.tensor_scalar(out=sel[:], in0=dd[:], scalar1=SEL_THR,
                                    scalar2=0.0, op0=ALU.is_gt)
            nc.sync.dma_start(out=sel_o[:], in_=sel[:])
            sumw = sp.tile([128, PPD], F32)
            nc.vector.tensor_copy(out=sumw[:], in_=s3[:, :, 2])
            nc.sync.dma_start(out=sumw_o[:], in_=sumw[:])
            a_t = sp.tile([128, PPD], F32)
            nc.sync.dma_start(out=a_t[:], in_=a_in[:])
            p2 = sp.tile([128, PPD], F32)
            nc.vector.tensor_tensor(out=p2[:], in0=sel[:], in1=a_t[:], op=ALU.mult)
            nc.sync.dma_start(out=pack2_o[:], in_=p2[:])
    nc.compile()
    return nc


# ---------------------------------------------------------------- L3: round B
def build_L3(KC):
    """Round B, block-structured: reuses L4's idxC/dstC tables; one-hot
    matmul segment sums replace scan+spill+boundary (saves 99 gathers)."""
    nc = bacc.Bacc("TRN2", target_bir_lowering=False, debug=False, num_devices=NC_N)
    tab = nc.dram_tensor("pack2_tab", [N_ALL + 1, 1], F32, kind="ExternalInput").ap()
    idxC = nc.dram_tensor("idxC", [NBLK, 128, KC], I32, kind="ExternalInput").ap()
    dstC = nc.dram_tensor("dstC", [NBLK, 128, KC], F32, kind="ExternalInput").ap()
    iota = nc.dram_tensor("iota", [128, 128], F32, kind="ExternalInput").ap()
    sumw_i = nc.dram_tensor("sumw_i", [128, NBLK], F32, kind="ExternalInput").ap()
    sel_i = nc.dram_tensor("sel_i", [128, NBLK], F32, kind="ExternalInput").ap()
    u_in = nc.dram_tensor("u_in", [NPAD, 64], F32, kind="ExternalInput").ap()
    u3_o = nc.dram_tensor("u3_o", [NPAD, 64], F32, kind="ExternalOutput").ap()

    OB = 8
    with tile.TileContext(nc) as tc:
        with tc.tile_pool(name="const", bufs=1) as cp, \
             tc.tile_pool(name="sb", bufs=3) as sp, \
             tc.tile_pool(name="sc", bufs=2) as scp, \
             tc.tile_pool(name="u", bufs=2) as up, \
             tc.tile_pool(name="ps", bufs=4, space="PSUM") as pp:
            iota_t = cp.tile([128, 128], F32)
            nc.sync.dma_start(out=iota_t[:], in_=iota[:])
            s2w = cp.tile([128, NBLK], F32)
            for b in range(NBLK):
                idx_t = sp.tile([128, KC], I32, tag="idx")
                nc.sync.dma_start(out=idx_t[:], in_=idxC[b])
                dst_t = sp.tile([128, KC], F32, tag="dst")
                nc.sync.dma_start(out=dst_t[:], in_=dstC[b])
                g = sp.tile([128, KC], F32, tag="g")
                for k in range(KC):
                    nc.gpsimd.indirect_dma_start(
                        out=g[:, k:k + 1], out_offset=None, in_=tab[:],
                        in_offset=bass.IndirectOffsetOnAxis(
                            ap=idx_t[:, k:k + 1], axis=0),
                    )
                S = sp.tile([128, KC * 128], F32, tag="S")
                Sv = S[:].rearrange("p (k d) -> p k d", d=128)
                for k0 in range(0, KC, OB):
                    kk = min(OB, KC - k0)
                    nc.vector.tensor_tensor(
                        out=Sv[:, k0:k0 + kk, :],
                        in0=dst_t[:, k0:k0 + kk].to_broadcast([128, kk, 128]),
                        in1=iota_t[:].unsqueeze(1).broadcast_to([128, kk, 128]),
                        op=ALU.is_equal)
                ps = pp.tile([128, 1], F32, tag="acc", space="PSUM")
                for k in range(KC):
                    nc.tensor.matmul(ps[:], lhsT=S[:, k * 128:(k + 1) * 128],
                                     rhs=g[:, k:k + 1],
                                     start=(k == 0), stop=(k == KC - 1))
                nc.vector.tensor_copy(out=s2w[:, b:b + 1], in_=ps[:])
            sumw_t = scp.tile([128, NBLK], F32, tag="sumw")
            nc.sync.dma_start(out=sumw_t[:], in_=sumw_i[:])
            zs = scp.tile([128, NBLK], F32, tag="zs")
            nc.vector.tensor_add(zs[:], s2w[:], sumw_t[:])
            wsel = scp.tile([128, NBLK], F32, tag="wsel")
            nc.scalar.activation(out=wsel[:], in_=zs[:], func=ACT.Sigmoid)
            sel_t = scp.tile([128, NBLK], F32, tag="sel")
            nc.sync.dma_start(out=sel_t[:], in_=sel_i[:])
            g2 = scp.tile([128, NBLK], F32, tag="g2")
            nc.vector.tensor_tensor(out=g2[:], in0=wsel[:], in1=sel_t[:],
                                    op=ALU.mult)
            STR = 14
            u_v = u_in.rearrange("(p j) f -> p j f", p=128)
            u3_v = u3_o.rearrange("(p j) f -> p j f", p=128)
            for s0 in range(0, NBLK, STR):
                ut = up.tile([128, STR * 64], F32, tag="ut")
                nc.sync.dma_start(out=ut[:], in_=u_v[:, s0:s0 + STR, :])
                u3t = up.tile([128, STR * 64], F32, tag="u3t")
                gb = g2[:, s0:s0 + STR].to_broadcast([128, STR, 64])
                nc.vector.tensor_tensor(
                    out=u3t[:].rearrange("p (j f) -> p j f", f=64),
                    in0=ut[:].rearrange("p (j f) -> p j f", f=64),
                    in1=gb, op=ALU.mult)
                nc.sync.dma_start(out=u3_v[:, s0:s0 + STR, :], in_=u3t[:])
    nc.compile()
    return nc


# ---------------------------------------------------------------- L4: round C
def build_L4(KC):
    nc = bacc.Bacc("TRN2", target_bir_lowering=False, debug=False, num_devices=NC_N)
    tab = nc.dram_tensor("u3_tab", [N_ALL, 64], F32, kind="ExternalInput").ap()
    u_in = nc.dram_tensor("u_in", [NPAD, 64], F32, kind="ExternalInput").ap()
    idxC = nc.dram_tensor("idxC", [NBLK, 128, KC], I32, kind="ExternalInput").ap()
    dstC = nc.dram_tensor("dstC", [NBLK, 128, KC], F32, kind="ExternalInput").ap()
    iota = nc.dram_tensor("iota", [128, 128], F32, kind="ExternalInput").ap()
    out_o = nc.dram_tensor("out_o", [NPAD, 64], F32, kind="ExternalOutput").ap()

    OB = 8  # one-hot batch (chunks per DVE op)
    with tile.TileContext(nc) as tc:
        with tc.tile_pool(name="const", bufs=1) as cp, \
             tc.tile_pool(name="sb", bufs=3) as sp, \
             tc.tile_pool(name="sc", bufs=3) as scp, \
             tc.tile_pool(name="ps", bufs=4, space="PSUM") as pp:
            iota_t = cp.tile([128, 128], F32)
            nc.sync.dma_start(out=iota_t[:], in_=iota[:])
            for b in range(NBLK):
                idx_t = sp.tile([128, KC], I32, tag="idx")
                nc.sync.dma_start(out=idx_t[:], in_=idxC[b])
                dst_t = sp.tile([128, KC], F32, tag="dst")
                nc.sync.dma_start(out=dst_t[:], in_=dstC[b])
                g = sp.tile([128, KC * 64], F32, tag="g")
                g3 = g[:].rearrange("p (k f) -> p k f", f=64)
                for k in range(KC):
                    nc.gpsimd.indirect_dma_start(
                        out=g3[:, k, :], out_offset=None, in_=tab[:],
                        in_offset=bass.IndirectOffsetOnAxis(ap=idx_t[:, k:k + 1], axis=0),
                    )
                S = sp.tile([128, KC * 128], F32, tag="S")
                Sv = S[:].rearrange("p (k d) -> p k d", d=128)
                for k0 in range(0, KC, OB):
                    kk = min(OB, KC - k0)
                    nc.vector.tensor_tensor(
                        out=Sv[:, k0:k0 + kk, :],
                        in0=dst_t[:, k0:k0 + kk].to_broadcast([128, kk, 128]),
                        in1=iota_t[:].unsqueeze(1).broadcast_to([128, kk, 128]),
                        op=ALU.is_equal)
                ps = pp.tile([128, 64], F32, tag="acc", space="PSUM")
                for k in range(KC):
                    nc.tensor.matmul(ps[:], lhsT=S[:, k * 128:(k + 1) * 128],
                                     rhs=g[:, k * 64:(k + 1) * 64],
                                     start=(k == 0), stop=(k == KC - 1))
                ut = sp.tile([128, 64], F32, tag="ut")
                nc.sync.dma_start(out=ut[:], in_=u_in[b * 128:(b + 1) * 128, :])
                ax = sp.tile([128, 64], F32, tag="ax")
                nc.vector.tensor_scalar_max(ax[:], ps[:], 0.0)
                o = sp.tile([128, 64], F32, tag="o")
                nc.vector.tensor_add(o[:], ut[:], ax[:])
                # expmap0 + proj
                sq = sp.tile([128, 64], F32, tag="sq")
                n2 = scp.tile([128, 1], F32, tag="n2")
                nc.scalar.activation(out=sq[:], in_=o[:], func=ACT.Square,
                                     accum_out=n2[:])
                nv = scp.tile([128, 1], F32, tag="nv")
                nc.scalar.activation(out=nv[:], in_=n2[:], func=ACT.Sqrt)
                nm = scp.tile([128, 1], F32, tag="nm")
                nc.vector.tensor_scalar_max(nm[:], nv[:], MIN_NORM)
                th = scp.tile([128, 1], F32, tag="th")
                nc.scalar.activation(out=th[:], in_=nm[:], func=ACT.Tanh)
                rn4 = scp.tile([128, 1], F32, tag="rn4")
                nc.vector.reciprocal(rn4[:], nm[:])
                f1 = scp.tile([128, 1], F32, tag="f1")
                nc.vector.tensor_tensor(out=f1[:], in0=th[:], in1=rn4[:],
                                        op=ALU.mult)
                # proj factor: min(maxn / tanh, 1)
                rt = scp.tile([128, 1], F32, tag="rt")
                nc.vector.reciprocal(rt[:], th[:])
                cap = scp.tile([128, 1], F32, tag="cap")
                nc.vector.tensor_scalar(out=cap[:], in0=rt[:], scalar1=PROJ_MAXN,
                                        scalar2=1.0, op0=ALU.mult, op1=ALU.min)
                f2 = scp.tile([128, 1], F32, tag="f2")
                nc.vector.tensor_tensor(out=f2[:], in0=f1[:], in1=cap[:],
                                        op=ALU.mult)
                oo = sp.tile([128, 64], F32, tag="oo")
                nc.vector.tensor_tensor(out=oo[:], in0=o[:],
                                        in1=f2[:].to_broadcast([128, 64]),
                                        op=ALU.mult)
                nc.sync.dma_start(out=out_o[b * 128:(b + 1) * 128, :], in_=oo[:])
    nc.compile()
    return nc


# ---------------------------------------------------------------- runner
def _run(nc, in_maps, trace):
    return bass_utils.run_bass_kernel_spmd(
        nc, in_maps, core_ids=list(range(NC_N)), trace=trace)


def kernel(x, edge_index, W_up, W_pl, W_lw, trace=None):
    if trace is None:
        trace = bool(int(os.environ.get("GNN_TRACE", "0")))
    if trace:
        bass_utils.upload_artifacts = lambda tmpdir: "/dev/null"

    x = np.asarray(x, np.float32)
    W_up = np.asarray(W_up, np.float32)
    W_pl = np.asarray(W_pl, np.float32)
    W_lw = np.asarray(W_lw, np.float32)
    cores, KA, KC, nodes_cp = host_prep(edge_index)
    exec_times = []

    # ---- L1
    Wcat = np.concatenate([W_pl, W_lw[64:128], W_lw[0:64]], axis=1)  # [64,4]
    xT_pad = np.zeros((NC_N, 128, NPAD), np.float32)
    for c in range(NC_N):
        xT_pad[c, :, :NSH] = x[nodes_cp[c]].T
    nc1 = build_L1()
    r1 = _run(nc1, [{"xT": xT_pad[c], "Wup": W_up, "Wcat": Wcat}
                    for c in range(NC_N)], trace)
    exec_times.append(r1.exec_time_ns)
    u_sh = [r1.results[c]["u_sh"] for c in range(NC_N)]
    pack_sh = [r1.results[c]["pack_sh"] for c in range(NC_N)]

    # ---- L2
    pack1_tab = np.concatenate(
        [np.concatenate([p[:, :3] for p in pack_sh], 0),
         np.zeros((1, 3), np.float32)], 0)
    nc2 = build_L2(KA)
    r2 = _run(nc2, [{"pack1_tab": pack1_tab,
                     "idxA": cores[c]["idxA"],
                     "bidx": cores[c]["bidx"],
                     "a_in": pack_sh[c][:, 3][cores[c]["sigmaA"]]}
                    for c in range(NC_N)], trace)
    exec_times.append(r2.exec_time_ns)
    sel = [r2.results[c]["sel_o"] for c in range(NC_N)]
    sumw = [r2.results[c]["sumw_o"] for c in range(NC_N)]
    pack2 = [r2.results[c]["pack2_o"] for c in range(NC_N)]

    # ---- L3  (block-structured; inputs/outputs in transposed-sigmaC layout)
    iota = np.tile(np.arange(128, dtype=np.float32)[None, :], (128, 1))
    p2_parts = []
    sigCT = []
    for c in range(NC_N):
        p2_full = np.zeros(NPAD, np.float32)
        p2_full[cores[c]["sigA_flat"]] = pack2[c].reshape(-1)
        p2_parts.append(p2_full)
        sigCT.append(cores[c]["sigC_flat"].reshape(NBLK, 128).T)  # [128, NBLK]
    pack2_tab = np.concatenate(
        [np.concatenate(p2_parts, 0), np.zeros(1, np.float32)], 0).reshape(-1, 1)

    def _toA(c, arr):  # sigmaA-layout [128, PPD] -> node vector
        full = np.zeros(NPAD, np.float32)
        full[cores[c]["sigA_flat"]] = arr.reshape(-1)
        return full

    nc3 = build_L3(KC)
    r3 = _run(nc3, [{"pack2_tab": pack2_tab,
                     "idxC": cores[c]["idxC"],
                     "dstC": cores[c]["dstC"],
                     "iota": iota,
                     "sumw_i": _toA(c, sumw[c])[sigCT[c]],
                     "sel_i": _toA(c, sel[c])[sigCT[c]],
                     "u_in": u_sh[c][sigCT[c].reshape(-1)]}
                    for c in range(NC_N)], trace)
    exec_times.append(r3.exec_time_ns)

    # ---- L4  (u3 rows come back in transposed-sigmaC order)
    u3_parts = []
    for c in range(NC_N):
        u3_full = np.zeros((NPAD, 64), np.float32)
        u3_full[sigCT[c].reshape(-1)] = r3.results[c]["u3_o"]
        u3_parts.append(u3_full)
    u3_tab = np.concatenate(u3_parts, 0)
    nc4 = build_L4(KC)
    r4 = _run(nc4, [{"u3_tab": u3_tab,
                     "u_in": u_sh[c][cores[c]["sigC_flat"]],
                     "idxC": cores[c]["idxC"],
                     "dstC": cores[c]["dstC"],
                     "iota": iota}
                    for c in range(NC_N)], trace)
    exec_times.append(r4.exec_time_ns)
    out = np.empty((NC_N * NSH, 64), np.float32)
    for c in range(NC_N):
        o_full = np.zeros((NPAD, 64), np.float32)
        o_full[cores[c]["sigC_flat"]] = r4.results[c]["out_o"]
        out[nodes_cp[c]] = o_full[:NSH]

    kernel.last_exec_times = exec_times
    return out

